# revision 10
# baseline (speedup 1.0000x reference)
"""Node2Node supervised-contrastive loss on 8 Trainium2 NeuronCores.

Strategy (data-parallel over the sample table):
  - The x table is split into 8 row-slices of N/8; core c owns slice c and
    normalizes it on-device into a bf16 "augmented" table [rows, 256] =
    [xn (128) | y (1) | zeros].
  - Every (anchor, sample) pair is routed (host-side index bookkeeping only)
    to the core owning the sampled row. Each core dma_gathers its pairs'
    rows, multiplies with the (device-normalized) anchor features, reduces
    over D with a binary tree on the vector engine, exponentiates, masks,
    and accumulates per-anchor partial numerator/denominator/count sums.
  - Pairs are laid out in "columns" of 128 (one per partition); anchors are
    grouped into 32 blocks of 128 slots so a column holds one sample of each
    of the block's anchors; per-anchor sums then become free-dim reductions.
  - A second tiny launch sums the 8 cores' per-anchor partials and computes
    -log(num/den)/cnt and the final scalar reduction on-device.
"""

import os
import sys

import numpy as np
import ml_dtypes

sys.path.insert(0, "/opt/trn_rl_repo")

import concourse.bass as bass
import concourse.bacc as bacc
import concourse.mybir as mybir
import concourse.tile as tile
from concourse import bass_utils

F32 = mybir.dt.float32
BF16 = mybir.dt.bfloat16
I16 = mybir.dt.int16
I32 = mybir.dt.int32
MUL = mybir.AluOpType.mult
ADD = mybir.AluOpType.add
SUB = mybir.AluOpType.subtract
EQ = mybir.AluOpType.is_equal
AFT = mybir.ActivationFunctionType


class CFG:
    def __init__(self, N=100000, D=128, A=4096, S=512, NC=8, TEMP=0.1, MT=56):
        self.N, self.D, self.A, self.S, self.NC, self.TEMP = N, D, A, S, NC, TEMP
        self.SL = N // NC                      # rows per slice
        self.NB = A // 128                     # anchor blocks (slots of 128)
        self.G = -(-self.SL // 128)            # slice col-groups of 128 rows
        self.SLP = self.G * 128                # padded slice rows
        self.MT = MT                           # max columns per gather call


REAL = CFG()


# --------------------------------------------------------------------------
# host-side index prep (pure numpy; integer bookkeeping only)
# --------------------------------------------------------------------------

def prep(cfg, x, y, anchors, sampled):
    N, A, S, NC, SL, NB = cfg.N, cfg.A, cfg.S, cfg.NC, cfg.SL, cfg.NB
    x = np.ascontiguousarray(np.asarray(x, dtype=np.float32))
    y64 = np.asarray(y, dtype=np.int64)
    anchors = np.asarray(anchors, dtype=np.int64)
    sampled = np.asarray(sampled, dtype=np.int64)

    core_of = sampled // SL                    # [A, S]
    # per (anchor, core) counts
    cnt = np.zeros((A, NC), dtype=np.int64)
    for c in range(NC):
        cnt[:, c] = (core_of == c).sum(1)

    # per-core anchor->slot permutation (sorted by count) and uniform block sizes
    perms, ranks = [], []
    Ms = np.zeros((NC, NB), dtype=np.int64)
    for c in range(NC):
        p = np.argsort(cnt[:, c], kind="stable")
        r = np.empty(A, dtype=np.int64)
        r[p] = np.arange(A)
        perms.append(p)
        ranks.append(r)
        Ms[c] = cnt[p, c].reshape(NB, 128).max(1)
    M = Ms.max(0)                              # uniform per-block columns
    Cj = np.concatenate([[0], np.cumsum(M)])   # block column offsets
    MTOT = int(Cj[-1])

    cores = []
    for c in range(NC):
        perm, rank = perms[c], ranks[c]
        a_list, s_list = np.nonzero(core_of == c)       # sorted by anchor
        local = (sampled[a_list, s_list] - c * SL).astype(np.int64)
        n = cnt[:, c]
        start = np.concatenate([[0], np.cumsum(n)])
        k = np.arange(len(a_list)) - start[a_list]      # within-anchor position
        r = rank[a_list]
        j, p = r // 128, r % 128
        col = Cj[j] + k
        idxmat = np.zeros((128, MTOT), dtype=np.int16)
        valid = np.zeros((128, MTOT), dtype=ml_dtypes.bfloat16)
        idxmat[p, col] = local.astype(np.int16)
        valid[p, col] = 1.0
        # flat gather list, column-major: position t = col*128 + p
        flat = idxmat.T.reshape(-1)                     # [MTOT*128]
        L = flat.size // 16
        wrapped = np.zeros((128, L), dtype=np.int16)
        w16 = flat.reshape(L, 16).T
        for g in range(8):
            wrapped[g * 16:(g + 1) * 16, :] = w16

        # anchor-side host data (slot order)
        aperm = anchors[perm]                           # [A] node ids, slot order
        xa = x[aperm].reshape(NB, 128, cfg.D).transpose(1, 0, 2).copy()  # [128,NB,D]
        win = (aperm // 32).astype(np.int16)
        Lw = A // 16
        wa = np.zeros((128, Lw), dtype=np.int16)
        ww = win.reshape(Lw, 16).T
        for g in range(8):
            wa[g * 16:(g + 1) * 16, :] = ww
        off = ((aperm % 32) * 2).astype(np.float32)
        off = off.reshape(NB, 128).T.copy()             # [128, NB]

        # slice inputs (padded)
        xs = np.ones((cfg.SLP, cfg.D), dtype=np.float32)
        xs[:SL] = x[c * SL:(c + 1) * SL]
        ysl = np.zeros((cfg.SLP, 2), dtype=np.int32)
        ysl[:SL] = y64[c * SL:(c + 1) * SL, None].view(np.int32).reshape(SL, 2)

        cores.append(dict(
            xs=xs, ys=ysl, xa=xa,
            yw=y64.view(np.int32).reshape(-1, 64),      # [N/32, 64] int32
            wa=wa, off=off, sidx=wrapped, vmask=valid,
        ))
    return cores, perms, M.astype(int).tolist()


# --------------------------------------------------------------------------
# kernel 1: per-core partial sums
# --------------------------------------------------------------------------

def build_k1(cfg, M, repeat=1):
    NB, D, G, SLP, MT = cfg.NB, cfg.D, cfg.G, cfg.SLP, cfg.MT
    MTOT = sum(M)
    WROWS = cfg.N // 32
    nc = bacc.Bacc("TRN2", target_bir_lowering=False, debug=False,
                   num_devices=cfg.NC, num_swdge_queues=4)
    xs = nc.dram_tensor("xs", [SLP, D], F32, kind="ExternalInput").ap()
    ys = nc.dram_tensor("ys", [SLP, 2], I32, kind="ExternalInput").ap()
    xa = nc.dram_tensor("xa", [128, NB, D], F32, kind="ExternalInput").ap()
    yw = nc.dram_tensor("yw", [WROWS, 64], I32, kind="ExternalInput").ap()
    wa = nc.dram_tensor("wa", [128, cfg.A // 16], I16, kind="ExternalInput").ap()
    off = nc.dram_tensor("off", [128, NB], F32, kind="ExternalInput").ap()
    sidx = nc.dram_tensor("sidx", [128, MTOT * 8], I16, kind="ExternalInput").ap()
    vmask = nc.dram_tensor("vmask", [128, MTOT], BF16, kind="ExternalInput").ap()
    acc_out = nc.dram_tensor("acc", [128, NB, 3], F32, kind="ExternalOutput").ap()

    with tile.TileContext(nc) as tc:
        with tc.tile_pool(name="dram", bufs=1, space="DRAM") as dpool:
            table = dpool.tile([SLP, 256], BF16)

            # ---- phase A: build normalized augmented slice table ----
            with tc.tile_pool(name="pa", bufs=2) as pa:
                half = (G + 1) // 2
                xsr = xs.rearrange("(g p) d -> p g d", p=128)
                ysr = ys.rearrange("(g p) k -> p g k", p=128)
                tbr = table[:].rearrange("(g p) e -> p g e", p=128)
                for h in range(2):
                    g0 = h * half
                    g1 = min(G, g0 + half)
                    gw = g1 - g0
                    if gw <= 0:
                        continue
                    xt = pa.tile([128, half, D], F32, tag="xt")
                    nc.sync.dma_start(xt[:, :gw, :], xsr[:, g0:g1, :])
                    sq = pa.tile([128, half, D], F32, tag="sq")
                    nc.vector.tensor_tensor(out=sq[:, :gw, :], in0=xt[:, :gw, :],
                                            in1=xt[:, :gw, :], op=MUL)
                    ss = pa.tile([128, half], F32, tag="ss")
                    nc.vector.reduce_sum(out=ss[:, :gw], in_=sq[:, :gw, :],
                                         axis=mybir.AxisListType.X)
                    nc.scalar.activation(ss[:, :gw], ss[:, :gw], AFT.Sqrt)
                    inv = pa.tile([128, half], F32, tag="inv")
                    nc.vector.reciprocal(inv[:, :gw], ss[:, :gw])
                    tb = pa.tile([128, half, 256], BF16, tag="tb")
                    nc.vector.tensor_tensor(
                        out=tb[:, :gw, 0:D], in0=xt[:, :gw, :],
                        in1=inv[:, :gw].unsqueeze(2).to_broadcast([128, gw, D]),
                        op=MUL)
                    yt = pa.tile([128, half, 2], I32, tag="yt")
                    nc.sync.dma_start(yt[:, :gw, :], ysr[:, g0:g1, :])
                    nc.vector.tensor_copy(out=tb[:, :gw, D:D + 1],
                                          in_=yt[:, :gw, 0:1])
                    nc.sync.dma_start(tbr[:, g0:g1, :], tb[:, :gw, :])

            # ---- phase B: anchor features + labels (slot layout) ----
            with tc.tile_pool(name="pb", bufs=1) as pb, \
                 tc.tile_pool(name="res", bufs=1) as res:
                xat = pb.tile([128, NB, D], F32)
                nc.sync.dma_start(xat[:], xa[:])
                sqa = pb.tile([128, NB, D], F32)
                nc.vector.tensor_tensor(out=sqa[:], in0=xat[:], in1=xat[:], op=MUL)
                ssa = pb.tile([128, NB], F32)
                nc.vector.reduce_sum(out=ssa[:], in_=sqa[:],
                                     axis=mybir.AxisListType.X)
                nc.scalar.activation(ssa[:], ssa[:], AFT.Sqrt)
                inva = pb.tile([128, NB], F32)
                nc.vector.reciprocal(inva[:], ssa[:])
                af = res.tile([128, NB, D], BF16)
                nc.vector.tensor_tensor(
                    out=af[:], in0=xat[:],
                    in1=inva[:].unsqueeze(2).to_broadcast([128, NB, D]), op=MUL)

                # anchor labels via 256B-window gather + one-hot select
                wat = pb.tile([128, cfg.A // 16], I16)
                nc.sync.dma_start(wat[:], wa[:])
                ywt = pb.tile([128, NB, 64], I32)
                nc.gpsimd.dma_gather(ywt[:], yw[:], wat[:], cfg.A, cfg.A, 64,
                                     single_packet=False)
                ywf = pb.tile([128, NB, 64], F32)
                nc.vector.tensor_copy(out=ywf[:], in_=ywt[:])
                ioti = pb.tile([128, 64], I32)
                nc.gpsimd.iota(ioti[:], pattern=[[1, 64]], base=0,
                               channel_multiplier=0)
                iot = pb.tile([128, 64], F32)
                nc.vector.tensor_copy(out=iot[:], in_=ioti[:])
                oft = pb.tile([128, NB], F32)
                nc.sync.dma_start(oft[:], off[:])
                oh = pb.tile([128, NB, 64], F32)
                nc.vector.tensor_tensor(
                    out=oh[:],
                    in0=iot[:].unsqueeze(1).to_broadcast([128, NB, 64]),
                    in1=oft[:].unsqueeze(2).to_broadcast([128, NB, 64]), op=EQ)
                ysel = pb.tile([128, NB, 64], F32)
                nc.vector.tensor_tensor(out=ysel[:], in0=ywf[:], in1=oh[:], op=MUL)
                ya = res.tile([128, NB], F32)
                nc.vector.reduce_sum(out=ya[:], in_=ysel[:],
                                     axis=mybir.AxisListType.X)
                acc = res.tile([128, NB, 3], F32)
                nc.vector.memset(acc[:], 0.0)

                # ---- phase C: main pair loop ----
                with tc.tile_pool(name="pcb", bufs=1) as pcb, \
                     tc.tile_pool(name="pc", bufs=2) as pc:
                  for _rep in range(repeat):
                    Cjs = [0] * NB
                    for j in range(1, NB):
                        Cjs[j] = Cjs[j - 1] + M[j - 1]
                    gq = 0
                    # biggest blocks first: the pipeline tail drains fastest
                    for j in sorted(range(NB), key=lambda jj: -M[jj]):
                        mj = M[j]
                        Cj = Cjs[j]
                        c0 = 0
                        while c0 < mj:
                            mt = min(MT, mj - c0)
                            col = Cj + c0            # global column offset
                            it = pc.tile([128, MT * 8], I16, tag=f"it{gq % 4}")
                            nc.sync.dma_start(
                                it[:, :mt * 8],
                                sidx[:, col * 8:(col + mt) * 8])
                            st = pcb.tile([128, MT, 256], BF16, tag=f"st{gq % 4}")
                            nc.gpsimd.dma_gather(
                                st[:, :mt, :], table[:], it[:, :mt * 8],
                                mt * 128, mt * 128, 256, single_packet=False,
                                queue_num=gq % 4)
                            gq += 1
                            ysd = pc.tile([128, MT], BF16, tag="ysd")
                            nc.scalar.activation(ysd[:, :mt], st[:, :mt, D],
                                                 AFT.Copy)
                            pr = st[:, :mt, D:2 * D]
                            nc.vector.tensor_tensor(
                                out=pr, in0=st[:, :mt, 0:D],
                                in1=af[:, j:j + 1, :].to_broadcast([128, mt, D]),
                                op=MUL)
                            w = D // 2
                            while w >= 1:
                                nc.vector.tensor_tensor(
                                    out=st[:, :mt, D:D + w],
                                    in0=st[:, :mt, D:D + w],
                                    in1=st[:, :mt, D + w:D + 2 * w], op=ADD)
                                w //= 2
                            e = pc.tile([128, MT], F32, tag="e")
                            nc.scalar.activation(e[:, :mt], st[:, :mt, D],
                                                 AFT.Exp, scale=1.0 / cfg.TEMP)
                            pm = pc.tile([128, MT], BF16, tag="pm")
                            nc.vector.tensor_tensor(
                                out=pm[:, :mt], in0=ysd[:, :mt],
                                in1=ya[:, j:j + 1].to_broadcast([128, mt]),
                                op=EQ)
                            vm = pc.tile([128, MT], BF16, tag="vm")
                            nc.sync.dma_start(vm[:, :mt], vmask[:, col:col + mt])
                            pmv = pc.tile([128, MT], BF16, tag="pmv")
                            nc.vector.tensor_tensor(out=pmv[:, :mt],
                                                    in0=pm[:, :mt],
                                                    in1=vm[:, :mt], op=MUL)
                            ev = pc.tile([128, MT], F32, tag="ev")
                            nc.vector.tensor_tensor(out=ev[:, :mt], in0=e[:, :mt],
                                                    in1=vm[:, :mt], op=MUL)
                            em = pc.tile([128, MT], F32, tag="em")
                            nc.vector.tensor_tensor(out=em[:, :mt],
                                                    in0=ev[:, :mt],
                                                    in1=pmv[:, :mt], op=MUL)
                            for q, src in ((0, em), (1, ev), (2, pmv)):
                                tmp = pc.tile([128, 1], F32, tag=f"tmp{q}")
                                nc.vector.reduce_sum(out=tmp[:], in_=src[:, :mt],
                                                     axis=mybir.AxisListType.X)
                                nc.vector.tensor_tensor(
                                    out=acc[:, j, q:q + 1], in0=acc[:, j, q:q + 1],
                                    in1=tmp[:], op=ADD)
                            c0 += mt
                nc.sync.dma_start(acc_out[:], acc[:])
    nc.compile()
    return nc


# --------------------------------------------------------------------------
# kernel 2: combine partials, per-anchor loss, total
# --------------------------------------------------------------------------

def build_k2(cfg):
    NB, NC = cfg.NB, cfg.NC
    nc = bacc.Bacc("TRN2", target_bir_lowering=False, debug=False, num_devices=1)
    parts = nc.dram_tensor("parts", [128, NC, NB, 3], F32,
                           kind="ExternalInput").ap()
    out = nc.dram_tensor("out", [1, 1], F32, kind="ExternalOutput").ap()
    with tile.TileContext(nc) as tc:
        with tc.tile_pool(name="p", bufs=1) as p, \
             tc.tile_pool(name="ps", bufs=1, space="PSUM") as psp:
            t = p.tile([128, NC, NB, 3], F32)
            nc.sync.dma_start(t[:], parts[:])
            s3 = p.tile([128, NB, 3], F32)
            # sum over the core axis (stride NB*3 innermost)
            tt = t[:].transpose([0, 2, 3, 1])
            nc.vector.reduce_sum(out=s3[:], in_=tt, axis=mybir.AxisListType.X)
            n_ = s3[:, :, 0]
            d_ = s3[:, :, 1]
            c_ = s3[:, :, 2]
            cz = p.tile([128, NB], F32)
            nc.vector.tensor_scalar(out=cz[:], in0=c_, scalar1=0.0, scalar2=None,
                                    op0=EQ)
            n1 = p.tile([128, NB], F32)
            nc.vector.tensor_tensor(out=n1[:], in0=n_, in1=cz[:], op=ADD)
            c1 = p.tile([128, NB], F32)
            nc.vector.tensor_scalar_max(out=c1[:], in0=c_, scalar1=1.0)
            lnn = p.tile([128, NB], F32)
            nc.scalar.activation(lnn[:], n1[:], AFT.Ln)
            lnd = p.tile([128, NB], F32)
            nc.scalar.activation(lnd[:], d_, AFT.Ln)
            df = p.tile([128, NB], F32)
            nc.vector.tensor_tensor(out=df[:], in0=lnd[:], in1=lnn[:], op=SUB)
            rc = p.tile([128, NB], F32)
            nc.vector.reciprocal(rc[:], c1[:])
            pa = p.tile([128, NB], F32)
            nc.vector.tensor_tensor(out=pa[:], in0=df[:], in1=rc[:], op=MUL)
            m = p.tile([128, NB], F32)
            nc.scalar.activation(m[:], cz[:], AFT.Copy, scale=-1.0, bias=1.0)
            pa2 = p.tile([128, NB], F32)
            nc.vector.tensor_tensor(out=pa2[:], in0=pa[:], in1=m[:], op=MUL)
            rs = p.tile([128, 1], F32)
            nc.vector.reduce_sum(out=rs[:], in_=pa2[:], axis=mybir.AxisListType.X)
            ones = p.tile([128, 1], F32)
            nc.vector.memset(ones[:], 1.0)
            acc = psp.tile([1, 1], F32)
            nc.tensor.matmul(out=acc[:], lhsT=rs[:], rhs=ones[:], start=True,
                             stop=True)
            res = p.tile([1, 1], F32)
            nc.vector.tensor_copy(out=res[:], in_=acc[:])
            nc.sync.dma_start(out[:], res[:])
    nc.compile()
    return nc


# --------------------------------------------------------------------------
# entry point
# --------------------------------------------------------------------------

def _run(cfg, x, y, anchors, sampled, trace=False):
    cores, perms, M = prep(cfg, x, y, anchors, sampled)
    nc1 = build_k1(cfg, M)
    in_maps = [dict(xs=c["xs"], ys=c["ys"], xa=c["xa"], yw=c["yw"],
                    wa=c["wa"], off=c["off"], sidx=c["sidx"], vmask=c["vmask"])
               for c in cores]
    kw = dict(trace=True, trace_cores=list(range(cfg.NC)), stitch_traces=False) \
        if trace else {}
    r1 = bass_utils.run_bass_kernel_spmd(nc1, in_maps,
                                         core_ids=list(range(cfg.NC)), **kw)
    # realign slot-order partials to anchor order (host: pure indexing)
    aligned = np.zeros((cfg.NC, cfg.A, 3), dtype=np.float32)
    for c in range(cfg.NC):
        acc = r1.results[c]["acc"]                       # [128, NB, 3]
        acc_t = acc.transpose(1, 0, 2).reshape(cfg.A, 3)  # slot-rank order
        aligned[c, perms[c]] = acc_t
    parts = aligned.reshape(cfg.NC, cfg.NB, 128, 3).transpose(2, 0, 1, 3).copy()
    nc2 = build_k2(cfg)
    r2 = bass_utils.run_bass_kernel_spmd(nc2, [dict(parts=parts)], core_ids=[0])
    val = np.float32(r2.results[0]["out"].reshape(-1)[0])
    return val, r1, aligned


def kernel(x, y, anchors, sampled):
    val, _, _ = _run(REAL, np.asarray(x), np.asarray(y), np.asarray(anchors),
                     np.asarray(sampled),
                     trace=os.environ.get("K_TRACE", "0") == "1")
    return np.asarray(val, dtype=np.float32)



# revision 11
# speedup vs baseline: 1.0271x; 1.0271x over previous
"""Node2Node supervised-contrastive loss on 8 Trainium2 NeuronCores.

Strategy (data-parallel over the sample table):
  - The x table is split into 8 row-slices of N/8; core c owns slice c and
    normalizes it on-device into a bf16 "augmented" table [rows, 256] =
    [xn (128) | y (1) | zeros].
  - Every (anchor, sample) pair is routed (host-side index bookkeeping only)
    to the core owning the sampled row. Each core dma_gathers its pairs'
    rows, multiplies with the (device-normalized) anchor features, reduces
    over D with a binary tree on the vector engine, exponentiates, masks,
    and accumulates per-anchor partial numerator/denominator/count sums.
  - Pairs are laid out in "columns" of 128 (one per partition); anchors are
    grouped into 32 blocks of 128 slots so a column holds one sample of each
    of the block's anchors; per-anchor sums then become free-dim reductions.
  - A second tiny launch sums the 8 cores' per-anchor partials and computes
    -log(num/den)/cnt and the final scalar reduction on-device.
"""

import os
import sys

import numpy as np
import ml_dtypes

sys.path.insert(0, "/opt/trn_rl_repo")

import concourse.bass as bass
import concourse.bacc as bacc
import concourse.mybir as mybir
import concourse.tile as tile
from concourse import bass_utils

F32 = mybir.dt.float32
BF16 = mybir.dt.bfloat16
I16 = mybir.dt.int16
I32 = mybir.dt.int32
MUL = mybir.AluOpType.mult
ADD = mybir.AluOpType.add
SUB = mybir.AluOpType.subtract
EQ = mybir.AluOpType.is_equal
AFT = mybir.ActivationFunctionType


class CFG:
    def __init__(self, N=100000, D=128, A=4096, S=512, NC=8, TEMP=0.1, MT=52):
        self.N, self.D, self.A, self.S, self.NC, self.TEMP = N, D, A, S, NC, TEMP
        self.SL = N // NC                      # rows per slice
        self.NB = A // 128                     # anchor blocks (slots of 128)
        self.G = -(-self.SL // 128)            # slice col-groups of 128 rows
        self.SLP = self.G * 128                # padded slice rows
        self.MT = MT                           # max columns per gather call


REAL = CFG()


# --------------------------------------------------------------------------
# host-side index prep (pure numpy; integer bookkeeping only)
# --------------------------------------------------------------------------

def prep(cfg, x, y, anchors, sampled):
    N, A, S, NC, SL, NB = cfg.N, cfg.A, cfg.S, cfg.NC, cfg.SL, cfg.NB
    x = np.ascontiguousarray(np.asarray(x, dtype=np.float32))
    y64 = np.asarray(y, dtype=np.int64)
    anchors = np.asarray(anchors, dtype=np.int64)
    sampled = np.asarray(sampled, dtype=np.int64)

    core_of = sampled // SL                    # [A, S]
    # per (anchor, core) counts
    cnt = np.zeros((A, NC), dtype=np.int64)
    for c in range(NC):
        cnt[:, c] = (core_of == c).sum(1)

    # per-core anchor->slot permutation (sorted by count) and uniform block sizes
    perms, ranks = [], []
    Ms = np.zeros((NC, NB), dtype=np.int64)
    for c in range(NC):
        p = np.argsort(cnt[:, c], kind="stable")
        r = np.empty(A, dtype=np.int64)
        r[p] = np.arange(A)
        perms.append(p)
        ranks.append(r)
        Ms[c] = cnt[p, c].reshape(NB, 128).max(1)
    M = Ms.max(0)                              # uniform per-block columns
    Cj = np.concatenate([[0], np.cumsum(M)])   # block column offsets
    MTOT = int(Cj[-1])

    cores = []
    for c in range(NC):
        perm, rank = perms[c], ranks[c]
        a_list, s_list = np.nonzero(core_of == c)       # sorted by anchor
        local = (sampled[a_list, s_list] - c * SL).astype(np.int64)
        n = cnt[:, c]
        start = np.concatenate([[0], np.cumsum(n)])
        k = np.arange(len(a_list)) - start[a_list]      # within-anchor position
        r = rank[a_list]
        j, p = r // 128, r % 128
        col = Cj[j] + k
        idxmat = np.zeros((128, MTOT), dtype=np.int16)
        valid = np.zeros((128, MTOT), dtype=ml_dtypes.bfloat16)
        idxmat[p, col] = local.astype(np.int16)
        valid[p, col] = 1.0
        # flat gather list, column-major: position t = col*128 + p
        flat = idxmat.T.reshape(-1)                     # [MTOT*128]
        L = flat.size // 16
        wrapped = np.zeros((128, L), dtype=np.int16)
        w16 = flat.reshape(L, 16).T
        for g in range(8):
            wrapped[g * 16:(g + 1) * 16, :] = w16

        # anchor-side host data (slot order)
        aperm = anchors[perm]                           # [A] node ids, slot order
        xa = x[aperm].reshape(NB, 128, cfg.D).transpose(1, 0, 2).copy()  # [128,NB,D]
        win = (aperm // 32).astype(np.int16)
        Lw = A // 16
        wa = np.zeros((128, Lw), dtype=np.int16)
        ww = win.reshape(Lw, 16).T
        for g in range(8):
            wa[g * 16:(g + 1) * 16, :] = ww
        off = ((aperm % 32) * 2).astype(np.float32)
        off = off.reshape(NB, 128).T.copy()             # [128, NB]

        # slice inputs (padded)
        xs = np.ones((cfg.SLP, cfg.D), dtype=np.float32)
        xs[:SL] = x[c * SL:(c + 1) * SL]
        ysl = np.zeros((cfg.SLP, 2), dtype=np.int32)
        ysl[:SL] = y64[c * SL:(c + 1) * SL, None].view(np.int32).reshape(SL, 2)

        cores.append(dict(
            xs=xs, ys=ysl, xa=xa,
            yw=y64.view(np.int32).reshape(-1, 64),      # [N/32, 64] int32
            wa=wa, off=off, sidx=wrapped, vmask=valid,
        ))
    return cores, perms, M.astype(int).tolist()


# --------------------------------------------------------------------------
# kernel 1: per-core partial sums
# --------------------------------------------------------------------------

def build_k1(cfg, M, repeat=1):
    NB, D, G, SLP, MT = cfg.NB, cfg.D, cfg.G, cfg.SLP, cfg.MT
    MTOT = sum(M)
    WROWS = cfg.N // 32
    nc = bacc.Bacc("TRN2", target_bir_lowering=False, debug=False,
                   num_devices=cfg.NC, num_swdge_queues=4)
    xs = nc.dram_tensor("xs", [SLP, D], F32, kind="ExternalInput").ap()
    ys = nc.dram_tensor("ys", [SLP, 2], I32, kind="ExternalInput").ap()
    xa = nc.dram_tensor("xa", [128, NB, D], F32, kind="ExternalInput").ap()
    yw = nc.dram_tensor("yw", [WROWS, 64], I32, kind="ExternalInput").ap()
    wa = nc.dram_tensor("wa", [128, cfg.A // 16], I16, kind="ExternalInput").ap()
    off = nc.dram_tensor("off", [128, NB], F32, kind="ExternalInput").ap()
    sidx = nc.dram_tensor("sidx", [128, MTOT * 8], I16, kind="ExternalInput").ap()
    vmask = nc.dram_tensor("vmask", [128, MTOT], BF16, kind="ExternalInput").ap()
    acc_out = nc.dram_tensor("acc", [128, NB, 3], F32, kind="ExternalOutput").ap()

    with tile.TileContext(nc) as tc:
        with tc.tile_pool(name="dram", bufs=1, space="DRAM") as dpool:
            table = dpool.tile([SLP, 256], BF16)

            # ---- phase A: build normalized augmented slice table ----
            with tc.tile_pool(name="pa", bufs=2) as pa:
                half = (G + 1) // 2
                xsr = xs.rearrange("(g p) d -> p g d", p=128)
                ysr = ys.rearrange("(g p) k -> p g k", p=128)
                tbr = table[:].rearrange("(g p) e -> p g e", p=128)
                for h in range(2):
                    g0 = h * half
                    g1 = min(G, g0 + half)
                    gw = g1 - g0
                    if gw <= 0:
                        continue
                    xt = pa.tile([128, half, D], F32, tag="xt")
                    nc.sync.dma_start(xt[:, :gw, :], xsr[:, g0:g1, :])
                    sq = pa.tile([128, half, D], F32, tag="sq")
                    nc.vector.tensor_tensor(out=sq[:, :gw, :], in0=xt[:, :gw, :],
                                            in1=xt[:, :gw, :], op=MUL)
                    ss = pa.tile([128, half], F32, tag="ss")
                    nc.vector.reduce_sum(out=ss[:, :gw], in_=sq[:, :gw, :],
                                         axis=mybir.AxisListType.X)
                    nc.scalar.activation(ss[:, :gw], ss[:, :gw], AFT.Sqrt)
                    inv = pa.tile([128, half], F32, tag="inv")
                    nc.vector.reciprocal(inv[:, :gw], ss[:, :gw])
                    tb = pa.tile([128, half, 256], BF16, tag="tb")
                    nc.vector.tensor_tensor(
                        out=tb[:, :gw, 0:D], in0=xt[:, :gw, :],
                        in1=inv[:, :gw].unsqueeze(2).to_broadcast([128, gw, D]),
                        op=MUL)
                    yt = pa.tile([128, half, 2], I32, tag="yt")
                    nc.sync.dma_start(yt[:, :gw, :], ysr[:, g0:g1, :])
                    nc.vector.tensor_copy(out=tb[:, :gw, D:D + 1],
                                          in_=yt[:, :gw, 0:1])
                    nc.sync.dma_start(tbr[:, g0:g1, :], tb[:, :gw, :])

            # ---- phase B: anchor features + labels (slot layout) ----
            with tc.tile_pool(name="pb", bufs=1) as pb, \
                 tc.tile_pool(name="res", bufs=1) as res:
                xat = pb.tile([128, NB, D], F32)
                nc.sync.dma_start(xat[:], xa[:])
                sqa = pb.tile([128, NB, D], F32)
                nc.vector.tensor_tensor(out=sqa[:], in0=xat[:], in1=xat[:], op=MUL)
                ssa = pb.tile([128, NB], F32)
                nc.vector.reduce_sum(out=ssa[:], in_=sqa[:],
                                     axis=mybir.AxisListType.X)
                nc.scalar.activation(ssa[:], ssa[:], AFT.Sqrt)
                inva = pb.tile([128, NB], F32)
                nc.vector.reciprocal(inva[:], ssa[:])
                af = res.tile([128, NB, D], BF16)
                nc.vector.tensor_tensor(
                    out=af[:], in0=xat[:],
                    in1=inva[:].unsqueeze(2).to_broadcast([128, NB, D]), op=MUL)

                # anchor labels via 256B-window gather + one-hot select
                wat = pb.tile([128, cfg.A // 16], I16)
                nc.sync.dma_start(wat[:], wa[:])
                ywt = pb.tile([128, NB, 64], I32)
                nc.gpsimd.dma_gather(ywt[:], yw[:], wat[:], cfg.A, cfg.A, 64,
                                     single_packet=False)
                ywf = pb.tile([128, NB, 64], F32)
                nc.vector.tensor_copy(out=ywf[:], in_=ywt[:])
                ioti = pb.tile([128, 64], I32)
                nc.gpsimd.iota(ioti[:], pattern=[[1, 64]], base=0,
                               channel_multiplier=0)
                iot = pb.tile([128, 64], F32)
                nc.vector.tensor_copy(out=iot[:], in_=ioti[:])
                oft = pb.tile([128, NB], F32)
                nc.sync.dma_start(oft[:], off[:])
                oh = pb.tile([128, NB, 64], F32)
                nc.vector.tensor_tensor(
                    out=oh[:],
                    in0=iot[:].unsqueeze(1).to_broadcast([128, NB, 64]),
                    in1=oft[:].unsqueeze(2).to_broadcast([128, NB, 64]), op=EQ)
                ysel = pb.tile([128, NB, 64], F32)
                nc.vector.tensor_tensor(out=ysel[:], in0=ywf[:], in1=oh[:], op=MUL)
                ya = res.tile([128, NB], F32)
                nc.vector.reduce_sum(out=ya[:], in_=ysel[:],
                                     axis=mybir.AxisListType.X)
                acc = res.tile([128, NB, 3], F32)
                nc.vector.memset(acc[:], 0.0)

                # ---- phase C: main pair loop ----
                with tc.tile_pool(name="pcb", bufs=1) as pcb, \
                     tc.tile_pool(name="pc", bufs=3) as pc:
                  for _rep in range(repeat):
                    Cj = 0
                    gq = 0
                    for j in range(NB):
                        mj = M[j]
                        c0 = 0
                        while c0 < mj:
                            mt = min(MT, mj - c0)
                            col = Cj + c0            # global column offset
                            it = pc.tile([128, MT * 8], I16, tag=f"it{gq % 4}")
                            nc.sync.dma_start(
                                it[:, :mt * 8],
                                sidx[:, col * 8:(col + mt) * 8])
                            st = pcb.tile([128, MT, 256], BF16, tag=f"st{gq % 4}")
                            nc.gpsimd.dma_gather(
                                st[:, :mt, :], table[:], it[:, :mt * 8],
                                mt * 128, mt * 128, 256, single_packet=False,
                                queue_num=gq % 4)
                            gq += 1
                            ysd = pc.tile([128, MT], BF16, tag="ysd")
                            nc.scalar.activation(ysd[:, :mt], st[:, :mt, D],
                                                 AFT.Copy)
                            pr = st[:, :mt, D:2 * D]
                            nc.vector.tensor_tensor(
                                out=pr, in0=st[:, :mt, 0:D],
                                in1=af[:, j:j + 1, :].to_broadcast([128, mt, D]),
                                op=MUL)
                            w = D // 2
                            while w >= 1:
                                nc.vector.tensor_tensor(
                                    out=st[:, :mt, D:D + w],
                                    in0=st[:, :mt, D:D + w],
                                    in1=st[:, :mt, D + w:D + 2 * w], op=ADD)
                                w //= 2
                            e = pc.tile([128, MT], F32, tag="e")
                            nc.scalar.activation(e[:, :mt], st[:, :mt, D],
                                                 AFT.Exp, scale=1.0 / cfg.TEMP)
                            pm = pc.tile([128, MT], BF16, tag="pm")
                            nc.vector.tensor_tensor(
                                out=pm[:, :mt], in0=ysd[:, :mt],
                                in1=ya[:, j:j + 1].to_broadcast([128, mt]),
                                op=EQ)
                            vm = pc.tile([128, MT], BF16, tag="vm")
                            nc.sync.dma_start(vm[:, :mt], vmask[:, col:col + mt])
                            pmv = pc.tile([128, MT], BF16, tag="pmv")
                            nc.vector.tensor_tensor(out=pmv[:, :mt],
                                                    in0=pm[:, :mt],
                                                    in1=vm[:, :mt], op=MUL)
                            ev = pc.tile([128, MT], F32, tag="ev")
                            nc.vector.tensor_tensor(out=ev[:, :mt], in0=e[:, :mt],
                                                    in1=vm[:, :mt], op=MUL)
                            em = pc.tile([128, MT], F32, tag="em")
                            nc.vector.tensor_tensor(out=em[:, :mt],
                                                    in0=ev[:, :mt],
                                                    in1=pmv[:, :mt], op=MUL)
                            for q, src in ((0, em), (1, ev), (2, pmv)):
                                tmp = pc.tile([128, 1], F32, tag=f"tmp{q}")
                                nc.vector.reduce_sum(out=tmp[:], in_=src[:, :mt],
                                                     axis=mybir.AxisListType.X)
                                nc.vector.tensor_tensor(
                                    out=acc[:, j, q:q + 1], in0=acc[:, j, q:q + 1],
                                    in1=tmp[:], op=ADD)
                            c0 += mt
                        Cj += mj
                nc.sync.dma_start(acc_out[:], acc[:])
    nc.compile()
    return nc


# --------------------------------------------------------------------------
# kernel 2: combine partials, per-anchor loss, total
# --------------------------------------------------------------------------

def build_k2(cfg):
    NB, NC = cfg.NB, cfg.NC
    nc = bacc.Bacc("TRN2", target_bir_lowering=False, debug=False, num_devices=1)
    parts = nc.dram_tensor("parts", [128, NC, NB, 3], F32,
                           kind="ExternalInput").ap()
    out = nc.dram_tensor("out", [1, 1], F32, kind="ExternalOutput").ap()
    with tile.TileContext(nc) as tc:
        with tc.tile_pool(name="p", bufs=1) as p, \
             tc.tile_pool(name="ps", bufs=1, space="PSUM") as psp:
            t = p.tile([128, NC, NB, 3], F32)
            nc.sync.dma_start(t[:], parts[:])
            s3 = p.tile([128, NB, 3], F32)
            # sum over the core axis (stride NB*3 innermost)
            tt = t[:].transpose([0, 2, 3, 1])
            nc.vector.reduce_sum(out=s3[:], in_=tt, axis=mybir.AxisListType.X)
            n_ = s3[:, :, 0]
            d_ = s3[:, :, 1]
            c_ = s3[:, :, 2]
            cz = p.tile([128, NB], F32)
            nc.vector.tensor_scalar(out=cz[:], in0=c_, scalar1=0.0, scalar2=None,
                                    op0=EQ)
            n1 = p.tile([128, NB], F32)
            nc.vector.tensor_tensor(out=n1[:], in0=n_, in1=cz[:], op=ADD)
            c1 = p.tile([128, NB], F32)
            nc.vector.tensor_scalar_max(out=c1[:], in0=c_, scalar1=1.0)
            lnn = p.tile([128, NB], F32)
            nc.scalar.activation(lnn[:], n1[:], AFT.Ln)
            lnd = p.tile([128, NB], F32)
            nc.scalar.activation(lnd[:], d_, AFT.Ln)
            df = p.tile([128, NB], F32)
            nc.vector.tensor_tensor(out=df[:], in0=lnd[:], in1=lnn[:], op=SUB)
            rc = p.tile([128, NB], F32)
            nc.vector.reciprocal(rc[:], c1[:])
            pa = p.tile([128, NB], F32)
            nc.vector.tensor_tensor(out=pa[:], in0=df[:], in1=rc[:], op=MUL)
            m = p.tile([128, NB], F32)
            nc.scalar.activation(m[:], cz[:], AFT.Copy, scale=-1.0, bias=1.0)
            pa2 = p.tile([128, NB], F32)
            nc.vector.tensor_tensor(out=pa2[:], in0=pa[:], in1=m[:], op=MUL)
            rs = p.tile([128, 1], F32)
            nc.vector.reduce_sum(out=rs[:], in_=pa2[:], axis=mybir.AxisListType.X)
            ones = p.tile([128, 1], F32)
            nc.vector.memset(ones[:], 1.0)
            acc = psp.tile([1, 1], F32)
            nc.tensor.matmul(out=acc[:], lhsT=rs[:], rhs=ones[:], start=True,
                             stop=True)
            res = p.tile([1, 1], F32)
            nc.vector.tensor_copy(out=res[:], in_=acc[:])
            nc.sync.dma_start(out[:], res[:])
    nc.compile()
    return nc


# --------------------------------------------------------------------------
# entry point
# --------------------------------------------------------------------------

def _run(cfg, x, y, anchors, sampled, trace=False):
    cores, perms, M = prep(cfg, x, y, anchors, sampled)
    nc1 = build_k1(cfg, M)
    in_maps = [dict(xs=c["xs"], ys=c["ys"], xa=c["xa"], yw=c["yw"],
                    wa=c["wa"], off=c["off"], sidx=c["sidx"], vmask=c["vmask"])
               for c in cores]
    kw = dict(trace=True, trace_cores=list(range(cfg.NC)), stitch_traces=False) \
        if trace else {}
    r1 = bass_utils.run_bass_kernel_spmd(nc1, in_maps,
                                         core_ids=list(range(cfg.NC)), **kw)
    # realign slot-order partials to anchor order (host: pure indexing)
    aligned = np.zeros((cfg.NC, cfg.A, 3), dtype=np.float32)
    for c in range(cfg.NC):
        acc = r1.results[c]["acc"]                       # [128, NB, 3]
        acc_t = acc.transpose(1, 0, 2).reshape(cfg.A, 3)  # slot-rank order
        aligned[c, perms[c]] = acc_t
    parts = aligned.reshape(cfg.NC, cfg.NB, 128, 3).transpose(2, 0, 1, 3).copy()
    nc2 = build_k2(cfg)
    r2 = bass_utils.run_bass_kernel_spmd(nc2, [dict(parts=parts)], core_ids=[0])
    val = np.float32(r2.results[0]["out"].reshape(-1)[0])
    return val, r1, aligned


def kernel(x, y, anchors, sampled):
    val, _, _ = _run(REAL, np.asarray(x), np.asarray(y), np.asarray(anchors),
                     np.asarray(sampled),
                     trace=os.environ.get("K_TRACE", "0") == "1")
    return np.asarray(val, dtype=np.float32)



# revision 17
# speedup vs baseline: 1.3106x; 1.2760x over previous
"""Node2Node supervised-contrastive loss on 8 Trainium2 NeuronCores.

Strategy (data-parallel over the sample table):
  - The x table is split into 8 row-slices of N/8; core c owns slice c and
    normalizes it on-device into a bf16 "augmented" table [rows, 256] =
    [xn (128) | y (1) | zeros].
  - Every (anchor, sample) pair is routed (host-side index bookkeeping only)
    to the core owning the sampled row. Each core dma_gathers its pairs'
    rows, multiplies with the (device-normalized) anchor features, reduces
    over D with a binary tree on the vector engine, exponentiates, masks,
    and accumulates per-anchor partial numerator/denominator/count sums.
  - Pairs are laid out in "columns" of 128 (one per partition); anchors are
    grouped into 32 blocks of 128 slots so a column holds one sample of each
    of the block's anchors; per-anchor sums then become free-dim reductions.
  - A second tiny launch sums the 8 cores' per-anchor partials and computes
    -log(num/den)/cnt and the final scalar reduction on-device.
"""

import os
import sys

import numpy as np
import ml_dtypes

sys.path.insert(0, "/opt/trn_rl_repo")

import concourse.bass as bass
import concourse.bacc as bacc
import concourse.mybir as mybir
import concourse.tile as tile
from concourse import bass_utils

F32 = mybir.dt.float32
BF16 = mybir.dt.bfloat16
I16 = mybir.dt.int16
I32 = mybir.dt.int32
MUL = mybir.AluOpType.mult
ADD = mybir.AluOpType.add
SUB = mybir.AluOpType.subtract
EQ = mybir.AluOpType.is_equal
AFT = mybir.ActivationFunctionType


class CFG:
    def __init__(self, N=100000, D=128, A=4096, S=512, NC=8, TEMP=0.1, MT=44):
        self.N, self.D, self.A, self.S, self.NC, self.TEMP = N, D, A, S, NC, TEMP
        self.SL = N // NC                      # rows per slice
        self.NB = A // 128                     # anchor blocks (slots of 128)
        self.G = -(-self.SL // 128)            # slice col-groups of 128 rows
        self.SLP = self.G * 128                # padded slice rows
        self.MT = MT                           # max columns per gather call


REAL = CFG()


# --------------------------------------------------------------------------
# host-side index prep (pure numpy; integer bookkeeping only)
# --------------------------------------------------------------------------

def prep(cfg, x, y, anchors, sampled):
    N, A, S, NC, SL, NB = cfg.N, cfg.A, cfg.S, cfg.NC, cfg.SL, cfg.NB
    x = np.ascontiguousarray(np.asarray(x, dtype=np.float32))
    y64 = np.asarray(y, dtype=np.int64)
    anchors = np.asarray(anchors, dtype=np.int64)
    sampled = np.asarray(sampled, dtype=np.int64)

    core_of = sampled // SL                    # [A, S]
    # per (anchor, core) counts
    cnt = np.zeros((A, NC), dtype=np.int64)
    for c in range(NC):
        cnt[:, c] = (core_of == c).sum(1)

    # per-core anchor->slot permutation (sorted by count) and uniform block sizes
    perms, ranks = [], []
    Ms = np.zeros((NC, NB), dtype=np.int64)
    for c in range(NC):
        p = np.argsort(cnt[:, c], kind="stable")
        r = np.empty(A, dtype=np.int64)
        r[p] = np.arange(A)
        perms.append(p)
        ranks.append(r)
        Ms[c] = cnt[p, c].reshape(NB, 128).max(1)
    M = Ms.max(0)                              # uniform per-block columns
    Cj = np.concatenate([[0], np.cumsum(M)])   # block column offsets
    MTOT = int(Cj[-1])

    cores = []
    for c in range(NC):
        perm, rank = perms[c], ranks[c]
        a_list, s_list = np.nonzero(core_of == c)       # sorted by anchor
        local = (sampled[a_list, s_list] - c * SL).astype(np.int64)
        n = cnt[:, c]
        start = np.concatenate([[0], np.cumsum(n)])
        k = np.arange(len(a_list)) - start[a_list]      # within-anchor position
        r = rank[a_list]
        j, p = r // 128, r % 128
        col = Cj[j] + k
        idxmat = np.zeros((128, MTOT), dtype=np.int16)
        valid = np.zeros((128, MTOT), dtype=ml_dtypes.bfloat16)
        idxmat[p, col] = local.astype(np.int16)
        valid[p, col] = 1.0
        # flat gather list, column-major: position t = col*128 + p
        flat = idxmat.T.reshape(-1)                     # [MTOT*128]
        L = flat.size // 16
        wrapped = np.zeros((128, L), dtype=np.int16)
        w16 = flat.reshape(L, 16).T
        for g in range(8):
            wrapped[g * 16:(g + 1) * 16, :] = w16

        # anchor-side host data (slot order)
        aperm = anchors[perm]                           # [A] node ids, slot order
        xa = x[aperm].reshape(NB, 128, cfg.D).transpose(1, 0, 2).copy()  # [128,NB,D]
        win = (aperm // 32).astype(np.int16)
        Lw = A // 16
        wa = np.zeros((128, Lw), dtype=np.int16)
        ww = win.reshape(Lw, 16).T
        for g in range(8):
            wa[g * 16:(g + 1) * 16, :] = ww
        off = ((aperm % 32) * 2).astype(np.float32)
        off = off.reshape(NB, 128).T.copy()             # [128, NB]

        # slice inputs (padded)
        xs = np.ones((cfg.SLP, cfg.D), dtype=np.float32)
        xs[:SL] = x[c * SL:(c + 1) * SL]
        ysl = np.zeros((cfg.SLP, 2), dtype=np.int32)
        ysl[:SL] = y64[c * SL:(c + 1) * SL, None].view(np.int32).reshape(SL, 2)

        cores.append(dict(
            xs=xs, ys=ysl, xa=xa,
            yw=y64.view(np.int32).reshape(-1, 64),      # [N/32, 64] int32
            wa=wa, off=off, sidx=wrapped, vmask=valid,
        ))
    return cores, perms, M.astype(int).tolist()


# --------------------------------------------------------------------------
# kernel 1: per-core partial sums
# --------------------------------------------------------------------------

def build_k1(cfg, M, repeat=1):
    NB, D, G, SLP, MT = cfg.NB, cfg.D, cfg.G, cfg.SLP, cfg.MT
    MTOT = sum(M)
    WROWS = cfg.N // 32
    nc = bacc.Bacc("TRN2", target_bir_lowering=False, debug=False,
                   num_devices=cfg.NC, num_swdge_queues=4)
    xs = nc.dram_tensor("xs", [SLP, D], F32, kind="ExternalInput").ap()
    ys = nc.dram_tensor("ys", [SLP, 2], I32, kind="ExternalInput").ap()
    xa = nc.dram_tensor("xa", [128, NB, D], F32, kind="ExternalInput").ap()
    yw = nc.dram_tensor("yw", [WROWS, 64], I32, kind="ExternalInput").ap()
    wa = nc.dram_tensor("wa", [128, cfg.A // 16], I16, kind="ExternalInput").ap()
    off = nc.dram_tensor("off", [128, NB], F32, kind="ExternalInput").ap()
    sidx = nc.dram_tensor("sidx", [128, MTOT * 8], I16, kind="ExternalInput").ap()
    vmask = nc.dram_tensor("vmask", [128, MTOT], BF16, kind="ExternalInput").ap()
    acc_out = nc.dram_tensor("acc", [128, NB, 3], F32, kind="ExternalOutput").ap()

    with tile.TileContext(nc) as tc:
        with tc.tile_pool(name="dram", bufs=1, space="DRAM") as dpool:
            table = dpool.tile([SLP, 256], BF16)

            # ---- phase A: build normalized augmented slice table ----
            with tc.tile_pool(name="pa", bufs=2) as pa:
                half = (G + 3) // 4
                xsr = xs.rearrange("(g p) d -> p g d", p=128)
                ysr = ys.rearrange("(g p) k -> p g k", p=128)
                tbr = table[:].rearrange("(g p) e -> p g e", p=128)
                for h in range(4):
                    g0 = h * half
                    g1 = min(G, g0 + half)
                    gw = g1 - g0
                    if gw <= 0:
                        continue
                    xt = pa.tile([128, half, D], F32, tag="xt")
                    nc.sync.dma_start(xt[:, :gw, :], xsr[:, g0:g1, :])
                    sq = pa.tile([128, half, D], F32, tag="sq")
                    nc.vector.tensor_tensor(out=sq[:, :gw, :], in0=xt[:, :gw, :],
                                            in1=xt[:, :gw, :], op=MUL)
                    ss = pa.tile([128, half], F32, tag="ss")
                    nc.vector.reduce_sum(out=ss[:, :gw], in_=sq[:, :gw, :],
                                         axis=mybir.AxisListType.X)
                    nc.scalar.activation(ss[:, :gw], ss[:, :gw], AFT.Sqrt)
                    inv = pa.tile([128, half], F32, tag="inv")
                    nc.vector.reciprocal(inv[:, :gw], ss[:, :gw])
                    tb = pa.tile([128, half, 256], BF16, tag="tb")
                    nc.vector.tensor_tensor(
                        out=tb[:, :gw, 0:D], in0=xt[:, :gw, :],
                        in1=inv[:, :gw].unsqueeze(2).to_broadcast([128, gw, D]),
                        op=MUL)
                    yt = pa.tile([128, half, 2], I32, tag="yt")
                    nc.sync.dma_start(yt[:, :gw, :], ysr[:, g0:g1, :])
                    nc.vector.tensor_copy(out=tb[:, :gw, D:D + 1],
                                          in_=yt[:, :gw, 0:1])
                    nc.sync.dma_start(tbr[:, g0:g1, :], tb[:, :gw, :])

            # ---- phase B: anchor features + labels (slot layout) ----
            with tc.tile_pool(name="pb", bufs=1) as pb, \
                 tc.tile_pool(name="res", bufs=1) as res:
                xat = pb.tile([128, NB, D], F32)
                nc.sync.dma_start(xat[:], xa[:])
                sqa = pb.tile([128, NB, D], F32)
                nc.vector.tensor_tensor(out=sqa[:], in0=xat[:], in1=xat[:], op=MUL)
                ssa = pb.tile([128, NB], F32)
                nc.vector.reduce_sum(out=ssa[:], in_=sqa[:],
                                     axis=mybir.AxisListType.X)
                nc.scalar.activation(ssa[:], ssa[:], AFT.Sqrt)
                inva = pb.tile([128, NB], F32)
                nc.vector.reciprocal(inva[:], ssa[:])
                af = res.tile([128, NB, D], BF16)
                nc.vector.tensor_tensor(
                    out=af[:], in0=xat[:],
                    in1=inva[:].unsqueeze(2).to_broadcast([128, NB, D]), op=MUL)

                # anchor labels via 256B-window gather + one-hot select
                wat = pb.tile([128, cfg.A // 16], I16)
                nc.sync.dma_start(wat[:], wa[:])
                ywt = pb.tile([128, NB, 64], I32)
                nc.gpsimd.dma_gather(ywt[:], yw[:], wat[:], cfg.A, cfg.A, 64,
                                     single_packet=False)
                ywf = pb.tile([128, NB, 64], F32)
                nc.vector.tensor_copy(out=ywf[:], in_=ywt[:])
                ioti = pb.tile([128, 64], I32)
                nc.gpsimd.iota(ioti[:], pattern=[[1, 64]], base=0,
                               channel_multiplier=0)
                iot = pb.tile([128, 64], F32)
                nc.vector.tensor_copy(out=iot[:], in_=ioti[:])
                oft = pb.tile([128, NB], F32)
                nc.sync.dma_start(oft[:], off[:])
                oh = pb.tile([128, NB, 64], F32)
                nc.vector.tensor_tensor(
                    out=oh[:],
                    in0=iot[:].unsqueeze(1).to_broadcast([128, NB, 64]),
                    in1=oft[:].unsqueeze(2).to_broadcast([128, NB, 64]), op=EQ)
                ysel = pb.tile([128, NB, 64], F32)
                nc.vector.tensor_tensor(out=ysel[:], in0=ywf[:], in1=oh[:], op=MUL)
                ya = res.tile([128, NB], F32)
                nc.vector.reduce_sum(out=ya[:], in_=ysel[:],
                                     axis=mybir.AxisListType.X)
                acc = res.tile([128, NB, 3], F32)
                nc.vector.memset(acc[:], 0.0)

                # ---- phase C: main pair loop ----
                with tc.tile_pool(name="pcb", bufs=1) as pcb, \
                     tc.tile_pool(name="pc", bufs=4) as pc:
                  for _rep in range(repeat):
                    Cj = 0
                    gq = 0
                    for j in range(NB):
                        mj = M[j]
                        c0 = 0
                        while c0 < mj:
                            mt = min(MT, mj - c0)
                            col = Cj + c0            # global column offset
                            it = pc.tile([128, MT * 8], I16, tag=f"it{gq % 5}")
                            nc.sync.dma_start(
                                it[:, :mt * 8],
                                sidx[:, col * 8:(col + mt) * 8])
                            st = pcb.tile([128, MT, 256], BF16, tag=f"st{gq % 5}")
                            nc.gpsimd.dma_gather(
                                st[:, :mt, :], table[:], it[:, :mt * 8],
                                mt * 128, mt * 128, 256, single_packet=False,
                                queue_num=gq % 4)
                            gq += 1
                            ysd = pc.tile([128, MT], BF16, tag="ysd")
                            nc.scalar.activation(ysd[:, :mt], st[:, :mt, D],
                                                 AFT.Copy)
                            pr = st[:, :mt, D:2 * D]
                            nc.vector.tensor_tensor(
                                out=pr, in0=st[:, :mt, 0:D],
                                in1=af[:, j:j + 1, :].to_broadcast([128, mt, D]),
                                op=MUL)
                            w = D // 2
                            while w >= 8:
                                nc.vector.tensor_tensor(
                                    out=st[:, :mt, D:D + w],
                                    in0=st[:, :mt, D:D + w],
                                    in1=st[:, :mt, D + w:D + 2 * w], op=ADD)
                                w //= 2
                            # final 8-wide fold into a separate tile frees st
                            # (the gather buffer) before the rest of the chain
                            sd = pc.tile([128, MT], F32, tag="sd")
                            nc.vector.reduce_sum(out=sd[:, :mt],
                                                 in_=st[:, :mt, D:D + 8],
                                                 axis=mybir.AxisListType.X)
                            e = pc.tile([128, MT], F32, tag="e")
                            nc.scalar.activation(e[:, :mt], sd[:, :mt],
                                                 AFT.Exp, scale=1.0 / cfg.TEMP)
                            pm = pc.tile([128, MT], BF16, tag="pm")
                            nc.vector.tensor_tensor(
                                out=pm[:, :mt], in0=ysd[:, :mt],
                                in1=ya[:, j:j + 1].to_broadcast([128, mt]),
                                op=EQ)
                            vm = pc.tile([128, MT], BF16, tag="vm")
                            nc.sync.dma_start(vm[:, :mt], vmask[:, col:col + mt])
                            pmv = pc.tile([128, MT], BF16, tag="pmv")
                            nc.vector.tensor_tensor(out=pmv[:, :mt],
                                                    in0=pm[:, :mt],
                                                    in1=vm[:, :mt], op=MUL)
                            ev = pc.tile([128, MT], F32, tag="ev")
                            nc.vector.tensor_tensor(out=ev[:, :mt], in0=e[:, :mt],
                                                    in1=vm[:, :mt], op=MUL)
                            em = pc.tile([128, MT], F32, tag="em")
                            nc.vector.tensor_tensor(out=em[:, :mt],
                                                    in0=ev[:, :mt],
                                                    in1=pmv[:, :mt], op=MUL)
                            for q, src in ((0, em), (1, ev), (2, pmv)):
                                tmp = pc.tile([128, 1], F32, tag=f"tmp{q}")
                                nc.vector.reduce_sum(out=tmp[:], in_=src[:, :mt],
                                                     axis=mybir.AxisListType.X)
                                nc.vector.tensor_tensor(
                                    out=acc[:, j, q:q + 1], in0=acc[:, j, q:q + 1],
                                    in1=tmp[:], op=ADD)
                            c0 += mt
                        Cj += mj
                nc.sync.dma_start(acc_out[:], acc[:])
    nc.compile()
    return nc


# --------------------------------------------------------------------------
# kernel 2: combine partials, per-anchor loss, total
# --------------------------------------------------------------------------

def build_k2(cfg):
    NB, NC = cfg.NB, cfg.NC
    nc = bacc.Bacc("TRN2", target_bir_lowering=False, debug=False, num_devices=1)
    parts = nc.dram_tensor("parts", [128, NC, NB, 3], F32,
                           kind="ExternalInput").ap()
    out = nc.dram_tensor("out", [1, 1], F32, kind="ExternalOutput").ap()
    with tile.TileContext(nc) as tc:
        with tc.tile_pool(name="p", bufs=1) as p, \
             tc.tile_pool(name="ps", bufs=1, space="PSUM") as psp:
            t = p.tile([128, NC, NB, 3], F32)
            nc.sync.dma_start(t[:], parts[:])
            s3 = p.tile([128, NB, 3], F32)
            # sum over the core axis (stride NB*3 innermost)
            tt = t[:].transpose([0, 2, 3, 1])
            nc.vector.reduce_sum(out=s3[:], in_=tt, axis=mybir.AxisListType.X)
            n_ = s3[:, :, 0]
            d_ = s3[:, :, 1]
            c_ = s3[:, :, 2]
            cz = p.tile([128, NB], F32)
            nc.vector.tensor_scalar(out=cz[:], in0=c_, scalar1=0.0, scalar2=None,
                                    op0=EQ)
            n1 = p.tile([128, NB], F32)
            nc.vector.tensor_tensor(out=n1[:], in0=n_, in1=cz[:], op=ADD)
            c1 = p.tile([128, NB], F32)
            nc.vector.tensor_scalar_max(out=c1[:], in0=c_, scalar1=1.0)
            lnn = p.tile([128, NB], F32)
            nc.scalar.activation(lnn[:], n1[:], AFT.Ln)
            lnd = p.tile([128, NB], F32)
            nc.scalar.activation(lnd[:], d_, AFT.Ln)
            df = p.tile([128, NB], F32)
            nc.vector.tensor_tensor(out=df[:], in0=lnd[:], in1=lnn[:], op=SUB)
            rc = p.tile([128, NB], F32)
            nc.vector.reciprocal(rc[:], c1[:])
            pa = p.tile([128, NB], F32)
            nc.vector.tensor_tensor(out=pa[:], in0=df[:], in1=rc[:], op=MUL)
            m = p.tile([128, NB], F32)
            nc.scalar.activation(m[:], cz[:], AFT.Copy, scale=-1.0, bias=1.0)
            pa2 = p.tile([128, NB], F32)
            nc.vector.tensor_tensor(out=pa2[:], in0=pa[:], in1=m[:], op=MUL)
            rs = p.tile([128, 1], F32)
            nc.vector.reduce_sum(out=rs[:], in_=pa2[:], axis=mybir.AxisListType.X)
            ones = p.tile([128, 1], F32)
            nc.vector.memset(ones[:], 1.0)
            acc = psp.tile([1, 1], F32)
            nc.tensor.matmul(out=acc[:], lhsT=rs[:], rhs=ones[:], start=True,
                             stop=True)
            res = p.tile([1, 1], F32)
            nc.vector.tensor_copy(out=res[:], in_=acc[:])
            nc.sync.dma_start(out[:], res[:])
    nc.compile()
    return nc


# --------------------------------------------------------------------------
# entry point
# --------------------------------------------------------------------------

def _run(cfg, x, y, anchors, sampled, trace=False):
    cores, perms, M = prep(cfg, x, y, anchors, sampled)
    nc1 = build_k1(cfg, M)
    in_maps = [dict(xs=c["xs"], ys=c["ys"], xa=c["xa"], yw=c["yw"],
                    wa=c["wa"], off=c["off"], sidx=c["sidx"], vmask=c["vmask"])
               for c in cores]
    kw = dict(trace=True, trace_cores=list(range(cfg.NC)), stitch_traces=False) \
        if trace else {}
    r1 = bass_utils.run_bass_kernel_spmd(nc1, in_maps,
                                         core_ids=list(range(cfg.NC)), **kw)
    # realign slot-order partials to anchor order (host: pure indexing)
    aligned = np.zeros((cfg.NC, cfg.A, 3), dtype=np.float32)
    for c in range(cfg.NC):
        acc = r1.results[c]["acc"]                       # [128, NB, 3]
        acc_t = acc.transpose(1, 0, 2).reshape(cfg.A, 3)  # slot-rank order
        aligned[c, perms[c]] = acc_t
    parts = aligned.reshape(cfg.NC, cfg.NB, 128, 3).transpose(2, 0, 1, 3).copy()
    nc2 = build_k2(cfg)
    r2 = bass_utils.run_bass_kernel_spmd(nc2, [dict(parts=parts)], core_ids=[0])
    val = np.float32(r2.results[0]["out"].reshape(-1)[0])
    return val, r1, aligned


def kernel(x, y, anchors, sampled):
    val, _, _ = _run(REAL, np.asarray(x), np.asarray(y), np.asarray(anchors),
                     np.asarray(sampled),
                     trace=os.environ.get("K_TRACE", "0") == "1")
    return np.asarray(val, dtype=np.float32)



# revision 21
# speedup vs baseline: 1.3958x; 1.0650x over previous
"""Node2Node supervised-contrastive loss on 8 Trainium2 NeuronCores.

Strategy (data-parallel over the sample table):
  - The x table is split into 8 row-slices of N/8; core c owns slice c and
    normalizes it on-device into a bf16 "augmented" table [rows, 256] =
    [xn (128) | y (1) | zeros].
  - Every (anchor, sample) pair is routed (host-side index bookkeeping only)
    to the core owning the sampled row. Each core dma_gathers its pairs'
    rows, multiplies with the (device-normalized) anchor features, reduces
    over D with a binary tree on the vector engine, exponentiates, masks,
    and accumulates per-anchor partial numerator/denominator/count sums.
  - Pairs are laid out in "columns" of 128 (one per partition); anchors are
    grouped into 32 blocks of 128 slots so a column holds one sample of each
    of the block's anchors; per-anchor sums then become free-dim reductions.
  - A second tiny launch sums the 8 cores' per-anchor partials and computes
    -log(num/den)/cnt and the final scalar reduction on-device.
"""

import os
import sys

import numpy as np
import ml_dtypes

sys.path.insert(0, "/opt/trn_rl_repo")

import concourse.bass as bass
import concourse.bacc as bacc
import concourse.mybir as mybir
import concourse.tile as tile
from concourse import bass_utils

F32 = mybir.dt.float32
BF16 = mybir.dt.bfloat16
I16 = mybir.dt.int16
I32 = mybir.dt.int32
MUL = mybir.AluOpType.mult
ADD = mybir.AluOpType.add
SUB = mybir.AluOpType.subtract
EQ = mybir.AluOpType.is_equal
AFT = mybir.ActivationFunctionType


class CFG:
    def __init__(self, N=100000, D=128, A=4096, S=512, NC=8, TEMP=0.1, MT=40):
        self.N, self.D, self.A, self.S, self.NC, self.TEMP = N, D, A, S, NC, TEMP
        self.SL = N // NC                      # rows per slice
        self.NB = A // 128                     # anchor blocks (slots of 128)
        self.G = -(-self.SL // 128)            # slice col-groups of 128 rows
        self.SLP = self.G * 128                # padded slice rows
        self.MT = MT                           # max columns per gather call


REAL = CFG()


# --------------------------------------------------------------------------
# host-side index prep (pure numpy; integer bookkeeping only)
# --------------------------------------------------------------------------

def prep(cfg, x, y, anchors, sampled):
    N, A, S, NC, SL, NB = cfg.N, cfg.A, cfg.S, cfg.NC, cfg.SL, cfg.NB
    x = np.ascontiguousarray(np.asarray(x, dtype=np.float32))
    y64 = np.asarray(y, dtype=np.int64)
    anchors = np.asarray(anchors, dtype=np.int64)
    sampled = np.asarray(sampled, dtype=np.int64)

    core_of = sampled // SL                    # [A, S]
    # per (anchor, core) counts
    cnt = np.zeros((A, NC), dtype=np.int64)
    for c in range(NC):
        cnt[:, c] = (core_of == c).sum(1)

    # per-core anchor->slot permutation (sorted by count) and uniform block sizes
    perms, ranks = [], []
    Ms = np.zeros((NC, NB), dtype=np.int64)
    for c in range(NC):
        p = np.argsort(cnt[:, c], kind="stable")
        r = np.empty(A, dtype=np.int64)
        r[p] = np.arange(A)
        perms.append(p)
        ranks.append(r)
        Ms[c] = cnt[p, c].reshape(NB, 128).max(1)
    M = Ms.max(0)                              # uniform per-block columns
    Cj = np.concatenate([[0], np.cumsum(M)])   # block column offsets
    MTOT = int(Cj[-1])

    cores = []
    for c in range(NC):
        perm, rank = perms[c], ranks[c]
        a_list, s_list = np.nonzero(core_of == c)       # sorted by anchor
        local = (sampled[a_list, s_list] - c * SL).astype(np.int64)
        n = cnt[:, c]
        start = np.concatenate([[0], np.cumsum(n)])
        k = np.arange(len(a_list)) - start[a_list]      # within-anchor position
        r = rank[a_list]
        j, p = r // 128, r % 128
        col = Cj[j] + k
        idxmat = np.zeros((128, MTOT), dtype=np.int16)
        valid = np.zeros((128, MTOT), dtype=ml_dtypes.bfloat16)
        idxmat[p, col] = local.astype(np.int16)
        valid[p, col] = 1.0
        # flat gather list, column-major: position t = col*128 + p
        flat = idxmat.T.reshape(-1)                     # [MTOT*128]
        L = flat.size // 16
        wrapped = np.zeros((128, L), dtype=np.int16)
        w16 = flat.reshape(L, 16).T
        for g in range(8):
            wrapped[g * 16:(g + 1) * 16, :] = w16

        # anchor-side host data (slot order)
        aperm = anchors[perm]                           # [A] node ids, slot order
        xa = x[aperm].reshape(NB, 128, cfg.D).transpose(1, 0, 2).copy()  # [128,NB,D]
        win = (aperm // 32).astype(np.int16)
        Lw = A // 16
        wa = np.zeros((128, Lw), dtype=np.int16)
        ww = win.reshape(Lw, 16).T
        for g in range(8):
            wa[g * 16:(g + 1) * 16, :] = ww
        off = ((aperm % 32) * 2).astype(np.float32)
        off = off.reshape(NB, 128).T.copy()             # [128, NB]

        # slice inputs (padded)
        xs = np.ones((cfg.SLP, cfg.D), dtype=np.float32)
        xs[:SL] = x[c * SL:(c + 1) * SL]
        ysl = np.zeros((cfg.SLP, 2), dtype=np.int32)
        ysl[:SL] = y64[c * SL:(c + 1) * SL, None].view(np.int32).reshape(SL, 2)

        cores.append(dict(
            xs=xs, ys=ysl, xa=xa,
            yw=y64.view(np.int32).reshape(-1, 64),      # [N/32, 64] int32
            wa=wa, off=off, sidx=wrapped, vmask=valid,
        ))
    return cores, perms, M.astype(int).tolist()


# --------------------------------------------------------------------------
# kernel 1: per-core partial sums
# --------------------------------------------------------------------------

def build_k1(cfg, M, repeat=1):
    NB, D, G, SLP, MT = cfg.NB, cfg.D, cfg.G, cfg.SLP, cfg.MT
    MTOT = sum(M)
    WROWS = cfg.N // 32
    nc = bacc.Bacc("TRN2", target_bir_lowering=False, debug=False,
                   num_devices=cfg.NC, num_swdge_queues=4)
    xs = nc.dram_tensor("xs", [SLP, D], F32, kind="ExternalInput").ap()
    ys = nc.dram_tensor("ys", [SLP, 2], I32, kind="ExternalInput").ap()
    xa = nc.dram_tensor("xa", [128, NB, D], F32, kind="ExternalInput").ap()
    yw = nc.dram_tensor("yw", [WROWS, 64], I32, kind="ExternalInput").ap()
    wa = nc.dram_tensor("wa", [128, cfg.A // 16], I16, kind="ExternalInput").ap()
    off = nc.dram_tensor("off", [128, NB], F32, kind="ExternalInput").ap()
    sidx = nc.dram_tensor("sidx", [128, MTOT * 8], I16, kind="ExternalInput").ap()
    vmask = nc.dram_tensor("vmask", [128, MTOT], BF16, kind="ExternalInput").ap()
    acc_out = nc.dram_tensor("acc", [128, NB, 3], F32, kind="ExternalOutput").ap()

    with tile.TileContext(nc) as tc:
        with tc.tile_pool(name="dram", bufs=1, space="DRAM") as dpool:
            table = dpool.tile([SLP, 256], BF16)

            # ---- phase A: build normalized augmented slice table ----
            with tc.tile_pool(name="pa", bufs=2) as pa:
                half = (G + 3) // 4
                xsr = xs.rearrange("(g p) d -> p g d", p=128)
                ysr = ys.rearrange("(g p) k -> p g k", p=128)
                tbr = table[:].rearrange("(g p) e -> p g e", p=128)
                for h in range(4):
                    g0 = h * half
                    g1 = min(G, g0 + half)
                    gw = g1 - g0
                    if gw <= 0:
                        continue
                    xt = pa.tile([128, half, D], F32, tag="xt")
                    nc.sync.dma_start(xt[:, :gw, :], xsr[:, g0:g1, :])
                    sq = pa.tile([128, half, D], F32, tag="sq")
                    nc.vector.tensor_tensor(out=sq[:, :gw, :], in0=xt[:, :gw, :],
                                            in1=xt[:, :gw, :], op=MUL)
                    ss = pa.tile([128, half], F32, tag="ss")
                    nc.vector.reduce_sum(out=ss[:, :gw], in_=sq[:, :gw, :],
                                         axis=mybir.AxisListType.X)
                    nc.scalar.activation(ss[:, :gw], ss[:, :gw], AFT.Sqrt)
                    inv = pa.tile([128, half], F32, tag="inv")
                    nc.vector.reciprocal(inv[:, :gw], ss[:, :gw])
                    tb = pa.tile([128, half, 256], BF16, tag="tb")
                    nc.vector.tensor_tensor(
                        out=tb[:, :gw, 0:D], in0=xt[:, :gw, :],
                        in1=inv[:, :gw].unsqueeze(2).to_broadcast([128, gw, D]),
                        op=MUL)
                    yt = pa.tile([128, half, 2], I32, tag="yt")
                    nc.sync.dma_start(yt[:, :gw, :], ysr[:, g0:g1, :])
                    nc.vector.tensor_copy(out=tb[:, :gw, D:D + 1],
                                          in_=yt[:, :gw, 0:1])
                    nc.sync.dma_start(tbr[:, g0:g1, :], tb[:, :gw, :])

            # ---- phase B: anchor features + labels (slot layout) ----
            with tc.tile_pool(name="pb", bufs=1) as pb, \
                 tc.tile_pool(name="res", bufs=1) as res:
                xat = pb.tile([128, NB, D], F32)
                nc.sync.dma_start(xat[:], xa[:])
                sqa = pb.tile([128, NB, D], F32)
                nc.vector.tensor_tensor(out=sqa[:], in0=xat[:], in1=xat[:], op=MUL)
                ssa = pb.tile([128, NB], F32)
                nc.vector.reduce_sum(out=ssa[:], in_=sqa[:],
                                     axis=mybir.AxisListType.X)
                nc.scalar.activation(ssa[:], ssa[:], AFT.Sqrt)
                inva = pb.tile([128, NB], F32)
                nc.vector.reciprocal(inva[:], ssa[:])
                af = res.tile([128, NB, D], BF16)
                nc.vector.tensor_tensor(
                    out=af[:], in0=xat[:],
                    in1=inva[:].unsqueeze(2).to_broadcast([128, NB, D]), op=MUL)

                # anchor labels via 256B-window gather + one-hot select
                wat = pb.tile([128, cfg.A // 16], I16)
                nc.sync.dma_start(wat[:], wa[:])
                ywt = pb.tile([128, NB, 64], I32)
                nc.gpsimd.dma_gather(ywt[:], yw[:], wat[:], cfg.A, cfg.A, 64,
                                     single_packet=False)
                ywf = pb.tile([128, NB, 64], F32)
                nc.vector.tensor_copy(out=ywf[:], in_=ywt[:])
                ioti = pb.tile([128, 64], I32)
                nc.gpsimd.iota(ioti[:], pattern=[[1, 64]], base=0,
                               channel_multiplier=0)
                iot = pb.tile([128, 64], F32)
                nc.vector.tensor_copy(out=iot[:], in_=ioti[:])
                oft = pb.tile([128, NB], F32)
                nc.sync.dma_start(oft[:], off[:])
                oh = pb.tile([128, NB, 64], F32)
                nc.vector.tensor_tensor(
                    out=oh[:],
                    in0=iot[:].unsqueeze(1).to_broadcast([128, NB, 64]),
                    in1=oft[:].unsqueeze(2).to_broadcast([128, NB, 64]), op=EQ)
                ysel = pb.tile([128, NB, 64], F32)
                nc.vector.tensor_tensor(out=ysel[:], in0=ywf[:], in1=oh[:], op=MUL)
                ya = res.tile([128, NB], F32)
                nc.vector.reduce_sum(out=ya[:], in_=ysel[:],
                                     axis=mybir.AxisListType.X)
                acc = res.tile([128, NB, 3], F32)
                nc.vector.memset(acc[:], 0.0)

                # ---- phase C: main pair loop ----
                with tc.tile_pool(name="pcb", bufs=1) as pcb, \
                     tc.tile_pool(name="pc", bufs=4) as pc:
                  for _rep in range(repeat):
                    Cj = 0
                    gq = 0
                    for j in range(NB):
                        mj = M[j]
                        c0 = 0
                        while c0 < mj:
                            mt = min(MT, mj - c0)
                            col = Cj + c0            # global column offset
                            it = pc.tile([128, MT * 8], I16, tag=f"it{gq % 6}")
                            nc.sync.dma_start(
                                it[:, :mt * 8],
                                sidx[:, col * 8:(col + mt) * 8])
                            st = pcb.tile([128, MT, 256], BF16, tag=f"st{gq % 6}")
                            nc.gpsimd.dma_gather(
                                st[:, :mt, :], table[:], it[:, :mt * 8],
                                mt * 128, mt * 128, 256, single_packet=False,
                                queue_num=gq % 4)
                            gq += 1
                            ysd = pc.tile([128, MT], BF16, tag="ysd")
                            nc.scalar.activation(ysd[:, :mt], st[:, :mt, D],
                                                 AFT.Copy)
                            pr = st[:, :mt, D:2 * D]
                            nc.vector.tensor_tensor(
                                out=pr, in0=st[:, :mt, 0:D],
                                in1=af[:, j:j + 1, :].to_broadcast([128, mt, D]),
                                op=MUL)
                            w = D // 2
                            while w >= 8:
                                nc.vector.tensor_tensor(
                                    out=st[:, :mt, D:D + w],
                                    in0=st[:, :mt, D:D + w],
                                    in1=st[:, :mt, D + w:D + 2 * w], op=ADD)
                                w //= 2
                            # final 8-wide fold into a separate tile frees st
                            # (the gather buffer) before the rest of the chain
                            sd = pc.tile([128, MT], F32, tag="sd")
                            nc.vector.reduce_sum(out=sd[:, :mt],
                                                 in_=st[:, :mt, D:D + 8],
                                                 axis=mybir.AxisListType.X)
                            e = pc.tile([128, MT], F32, tag="e")
                            nc.scalar.activation(e[:, :mt], sd[:, :mt],
                                                 AFT.Exp, scale=1.0 / cfg.TEMP)
                            pm = pc.tile([128, MT], BF16, tag="pm")
                            nc.vector.tensor_tensor(
                                out=pm[:, :mt], in0=ysd[:, :mt],
                                in1=ya[:, j:j + 1].to_broadcast([128, mt]),
                                op=EQ)
                            vm = pc.tile([128, MT], BF16, tag="vm")
                            nc.sync.dma_start(vm[:, :mt], vmask[:, col:col + mt])
                            pmv = pc.tile([128, MT], BF16, tag="pmv")
                            nc.vector.tensor_tensor(out=pmv[:, :mt],
                                                    in0=pm[:, :mt],
                                                    in1=vm[:, :mt], op=MUL)
                            ev = pc.tile([128, MT], F32, tag="ev")
                            nc.vector.tensor_tensor(out=ev[:, :mt], in0=e[:, :mt],
                                                    in1=vm[:, :mt], op=MUL)
                            em = pc.tile([128, MT], F32, tag="em")
                            nc.vector.tensor_tensor(out=em[:, :mt],
                                                    in0=ev[:, :mt],
                                                    in1=pmv[:, :mt], op=MUL)
                            for q, src in ((0, em), (1, ev), (2, pmv)):
                                tmp = pc.tile([128, 1], F32, tag=f"tmp{q}")
                                nc.vector.reduce_sum(out=tmp[:], in_=src[:, :mt],
                                                     axis=mybir.AxisListType.X)
                                nc.vector.tensor_tensor(
                                    out=acc[:, j, q:q + 1], in0=acc[:, j, q:q + 1],
                                    in1=tmp[:], op=ADD)
                            c0 += mt
                        Cj += mj
                nc.sync.dma_start(acc_out[:], acc[:])
    nc.compile()
    return nc


# --------------------------------------------------------------------------
# kernel 2: combine partials, per-anchor loss, total
# --------------------------------------------------------------------------

def build_k2(cfg):
    NB, NC = cfg.NB, cfg.NC
    nc = bacc.Bacc("TRN2", target_bir_lowering=False, debug=False, num_devices=1)
    parts = nc.dram_tensor("parts", [128, NC, NB, 3], F32,
                           kind="ExternalInput").ap()
    out = nc.dram_tensor("out", [1, 1], F32, kind="ExternalOutput").ap()
    with tile.TileContext(nc) as tc:
        with tc.tile_pool(name="p", bufs=1) as p, \
             tc.tile_pool(name="ps", bufs=1, space="PSUM") as psp:
            t = p.tile([128, NC, NB, 3], F32)
            nc.sync.dma_start(t[:], parts[:])
            s3 = p.tile([128, NB, 3], F32)
            # sum over the core axis (stride NB*3 innermost)
            tt = t[:].transpose([0, 2, 3, 1])
            nc.vector.reduce_sum(out=s3[:], in_=tt, axis=mybir.AxisListType.X)
            n_ = s3[:, :, 0]
            d_ = s3[:, :, 1]
            c_ = s3[:, :, 2]
            cz = p.tile([128, NB], F32)
            nc.vector.tensor_scalar(out=cz[:], in0=c_, scalar1=0.0, scalar2=None,
                                    op0=EQ)
            n1 = p.tile([128, NB], F32)
            nc.vector.tensor_tensor(out=n1[:], in0=n_, in1=cz[:], op=ADD)
            c1 = p.tile([128, NB], F32)
            nc.vector.tensor_scalar_max(out=c1[:], in0=c_, scalar1=1.0)
            lnn = p.tile([128, NB], F32)
            nc.scalar.activation(lnn[:], n1[:], AFT.Ln)
            lnd = p.tile([128, NB], F32)
            nc.scalar.activation(lnd[:], d_, AFT.Ln)
            df = p.tile([128, NB], F32)
            nc.vector.tensor_tensor(out=df[:], in0=lnd[:], in1=lnn[:], op=SUB)
            rc = p.tile([128, NB], F32)
            nc.vector.reciprocal(rc[:], c1[:])
            pa = p.tile([128, NB], F32)
            nc.vector.tensor_tensor(out=pa[:], in0=df[:], in1=rc[:], op=MUL)
            m = p.tile([128, NB], F32)
            nc.scalar.activation(m[:], cz[:], AFT.Copy, scale=-1.0, bias=1.0)
            pa2 = p.tile([128, NB], F32)
            nc.vector.tensor_tensor(out=pa2[:], in0=pa[:], in1=m[:], op=MUL)
            rs = p.tile([128, 1], F32)
            nc.vector.reduce_sum(out=rs[:], in_=pa2[:], axis=mybir.AxisListType.X)
            ones = p.tile([128, 1], F32)
            nc.vector.memset(ones[:], 1.0)
            acc = psp.tile([1, 1], F32)
            nc.tensor.matmul(out=acc[:], lhsT=rs[:], rhs=ones[:], start=True,
                             stop=True)
            res = p.tile([1, 1], F32)
            nc.vector.tensor_copy(out=res[:], in_=acc[:])
            nc.sync.dma_start(out[:], res[:])
    nc.compile()
    return nc


# --------------------------------------------------------------------------
# entry point
# --------------------------------------------------------------------------

def _run(cfg, x, y, anchors, sampled, trace=False):
    cores, perms, M = prep(cfg, x, y, anchors, sampled)
    nc1 = build_k1(cfg, M)
    in_maps = [dict(xs=c["xs"], ys=c["ys"], xa=c["xa"], yw=c["yw"],
                    wa=c["wa"], off=c["off"], sidx=c["sidx"], vmask=c["vmask"])
               for c in cores]
    kw = dict(trace=True, trace_cores=[0], stitch_traces=False) \
        if trace else {}
    r1 = bass_utils.run_bass_kernel_spmd(nc1, in_maps,
                                         core_ids=list(range(cfg.NC)), **kw)
    # realign slot-order partials to anchor order (host: pure indexing)
    aligned = np.zeros((cfg.NC, cfg.A, 3), dtype=np.float32)
    for c in range(cfg.NC):
        acc = r1.results[c]["acc"]                       # [128, NB, 3]
        acc_t = acc.transpose(1, 0, 2).reshape(cfg.A, 3)  # slot-rank order
        aligned[c, perms[c]] = acc_t
    parts = aligned.reshape(cfg.NC, cfg.NB, 128, 3).transpose(2, 0, 1, 3).copy()
    nc2 = build_k2(cfg)
    r2 = bass_utils.run_bass_kernel_spmd(nc2, [dict(parts=parts)], core_ids=[0])
    val = np.float32(r2.results[0]["out"].reshape(-1)[0])
    return val, r1, aligned


def kernel(x, y, anchors, sampled):
    val, _, _ = _run(REAL, np.asarray(x), np.asarray(y), np.asarray(anchors),
                     np.asarray(sampled),
                     trace=os.environ.get("K_TRACE", "0") == "1")
    return np.asarray(val, dtype=np.float32)

